# revision 7
# baseline (speedup 1.0000x reference)
"""Trainium2 Bass kernel for nn_Bessel: out = i0e(z) * exp(z - 2a), z = 2a*sqrt((1+x@yT)/2), a=10.

Math: out = exp(z - 20 + ln i0e(z)) = exp(t(z) - 20), t(z) = z + ln i0e(z).
With unit-norm rows, z = sqrt(200*c + 200) for c = x@yT in [-0.726, 0.816],
so z lies in [7.4, 19.1].  Key identity used here ("sqs" mode):

  t(z) ~= A + B*sqrt(z^2 + delta)        max rel err 2.05e-3 on [7.2, 19.3]

Since z^2 = 200c + 200 is linear in c, the whole correction folds into the
Sqrt activation's free affine:  w = Sqrt(200*c + (200+delta)), out =
Exp(B*w + (A-20)).  Exactly two ACT passes per element, no DVE pass.

Matmul: a single fp16 x fp16 matmul (fp16's 11-bit mantissa on unit-norm
data is accurate enough: ~3e-4 out rel err) replaces the bf16 hi/lo split
(which needed 2 matmuls with alternating stationary operands; measured
~467-553 ns per 512-col matmul from per-instruction LDWEIGHTS+dispatch
overhead -> 120-142 us of PE time, the previous bottleneck).  Now: 128
matmuls of K=64, N=512 (psum-bank limit; N=1024 fails an ISA check) with
one stationary per M-tile -> ~59 us PE, fully hidden under ACT.

Schedule per core (row-shard of x, y replicated; no collectives):
  Phase A (sqrt table): per 128x2048 PSUM chunk (2 bufs): 4x PE fp16
      matmul -> ACT Sqrt evac direct to fp16 w tiles (all 8 M-tiles in SBUF)
  Phase B (exp table):  per M-tile: ACT Exp -> bf16 out tile -> DMA to HBM
  One table switch per phase; bf16 output halves the HBM write (16 MB/core,
  upcast to fp32 on host).  obf_bufs=3: out-DMA runs ~8us per 2MB tile
  (~250 GB/s effective), so 2 bufs couple the Exps to DMA drain; the 3rd
  lets the trailing DMAs spill into the next iteration's phase A.

Error budget (numpy-simulated, seed-0 inputs; HW matches): L2 rel 4.43e-3
(fit 1.2e-3 + fp16 w 3.9e-3 + bf16 out 1.1e-3 + fp16 mm 0.3e-3), gate 2e-2.
ACT busy floor: 32 sqrt x ~2.08us + 8 exp x ~7.01us + 2 table loads
  = 66.6 + 56.1 + 2.6 = ~125us; measured (For_i differential, median of 3):
  ~129 us vs 210 us baseline.  Micro-benched: phase A 65.7us (ACT-paced),
  phase B 56us ACT + DMA tail; engines at floor, no further headroom found
  (DVE has no sqrt/exp and is slower than ACT per element; custom ACT
  tables would need a firmware rebuild).
"""

import contextlib

import numpy as np

import concourse.bacc as bacc
import concourse.mybir as mybir
from concourse.tile import TileContext
from concourse.tile_autobufs import add_dep_helper
from concourse.bass_utils import run_bass_kernel_spmd

AF = mybir.ActivationFunctionType
F32 = mybir.dt.float32
F16 = mybir.dt.float16
BF16 = mybir.dt.bfloat16

N_CORES = 8
N_ROWS, M_COLS, DIM = 8192, 8192, 64
ROWS = N_ROWS // N_CORES          # 1024 rows of x per core
MTILES = ROWS // 128              # 8 partition tiles per core
PSUM_FD = 2048                    # 4 PSUM banks per psum tile, 2 bufs
MM_N = 512                        # moving free dim (psum bank limit)

# minimax fit of t(z) = z + ln(i0e(z)) ~= A + B*sqrt(z^2 + delta), z in [7.2, 19.3]
SQS_A = -2.18471144825
SQS_B = 0.980389112036
SQS_D = 6.67629017188
SQ_BIAS = 200.0 + SQS_D           # Sqrt(200*c + SQ_BIAS)
EXP_SCALE = SQS_B
EXP_BIAS = SQS_A - 20.0

MODE = "sqs"

_cache = {}


def _build(mode, iters=1, psum_fd=PSUM_FD, exp_split=1, obf_bufs=3, mm_n=MM_N):
    assert mode == "sqs"
    nc = bacc.Bacc(None, target_bir_lowering=False)
    xq_d = nc.dram_tensor("xq", [DIM, ROWS], F16, kind="ExternalInput")
    yq_d = nc.dram_tensor("yq", [DIM, M_COLS], F16, kind="ExternalInput")
    out_d = nc.dram_tensor("out", [ROWS, M_COLS], BF16, kind="ExternalOutput")

    with TileContext(nc) as tc:
        with (
            tc.tile_pool(name="inp", bufs=1) as inp,
            tc.tile_pool(name="consts", bufs=1) as consts,
            tc.tile_pool(name="zw", bufs=MTILES) as zwpool,
            tc.tile_pool(name="obf", bufs=obf_bufs) as obfpool,
            tc.tile_pool(name="psum", bufs=4096 // psum_fd, space="PSUM") as psum,
        ):
            xq = inp.tile([DIM, ROWS], F16)
            yq = inp.tile([DIM, M_COLS], F16)
            nc.sync.dma_start(out=xq[:], in_=xq_d[:])
            for q in range(0, M_COLS, 2048):
                nc.sync.dma_start(out=yq[:, q:q + 2048], in_=yq_d[:, q:q + 2048])

            bsq = consts.tile([128, 1], F32)
            nc.gpsimd.memset(bsq[:], float(SQ_BIAS))
            bexp = consts.tile([128, 1], F32)
            nc.gpsimd.memset(bexp[:], float(EXP_BIAS))

            nchunk = M_COLS // psum_fd
            loop_cm = tc.For_i(0, iters) if iters > 1 else contextlib.nullcontext(0)
            with loop_cm as _i:
                zw_tiles = {}
                last_evac = None
                for m in range(MTILES):
                    zw = zwpool.tile([128, M_COLS], F16, tag="zw")
                    zw_tiles[m] = zw
                    msl = slice(m * 128, (m + 1) * 128)
                    for nb in range(nchunk):
                        pt = psum.tile([128, psum_fd], F32, tag="ps")
                        for j in range(psum_fd // mm_n):
                            col = nb * psum_fd + j * mm_n
                            nc.tensor.matmul(
                                pt[:, j * mm_n:(j + 1) * mm_n],
                                xq[:, msl], yq[:, col:col + mm_n],
                                start=True, stop=True,
                            )
                        sl = slice(nb * psum_fd, (nb + 1) * psum_fd)
                        # w = sqrt(200*c + 200 + delta), written as fp16
                        last_evac = nc.scalar.activation(
                            zw[:, sl], pt[:], AF.Sqrt, bias=bsq[:], scale=200.0
                        )
                for m in range(MTILES):
                    zw = zw_tiles[m]
                    efd = M_COLS // exp_split
                    obf = obfpool.tile([128, M_COLS], BF16, tag="obf")
                    for e in range(exp_split):
                        esl = slice(e * efd, (e + 1) * efd)
                        exp_inst = nc.scalar.activation(
                            obf[:, esl], zw[:, esl], AF.Exp,
                            bias=bexp[:], scale=float(EXP_SCALE)
                        )
                        # keep every Exp behind the last Sqrt evac so the ACT
                        # table is switched exactly twice per iteration
                        add_dep_helper(
                            exp_inst.ins, last_evac.ins, sync=False,
                            reason="batch exps after all sqrts (table switch)",
                        )
                        nc.sync.dma_start(
                            out=out_d[m * 128:(m + 1) * 128, esl], in_=obf[:, esl]
                        )

    nc.finalize()
    return nc


LAST_RESULTS = None


def _prep_inputs(x, y):
    """FULL fp32 x, y -> per-core input maps (fp16, transposed)."""
    yq = np.ascontiguousarray(y.T.astype(np.float16))
    in_maps = []
    for i in range(N_CORES):
        xq = np.ascontiguousarray(x[i * ROWS:(i + 1) * ROWS].T.astype(np.float16))
        in_maps.append({"xq": xq, "yq": yq})
    return in_maps


def kernel(x: np.ndarray, y: np.ndarray) -> np.ndarray:
    global LAST_RESULTS
    x = np.ascontiguousarray(x, dtype=np.float32)
    y = np.ascontiguousarray(y, dtype=np.float32)
    assert x.shape == (N_ROWS, DIM) and y.shape == (M_COLS, DIM)

    if MODE not in _cache:
        _cache[MODE] = _build(MODE)
    nc = _cache[MODE]

    in_maps = _prep_inputs(x, y)
    LAST_RESULTS = run_bass_kernel_spmd(nc, in_maps, list(range(N_CORES)))
    out = np.concatenate([r["out"] for r in LAST_RESULTS.results], axis=0)
    if out.dtype != np.float32:
        out = out.astype(np.float32)
    return out
